# revision 1
# baseline (speedup 1.0000x reference)
"""CTC loss (keras ctc_batch_cost semantics) on 8 Trainium2 NeuronCores.

Strategy: pure data parallel, batch 512 = 8 cores x 64 examples. The CTC
forward DP runs in LINEAR probability space with an exponential tilt
(every state-advance weighted g=1/4; path-independent so it cancels in
the fwd*bwd combine), and TWO DP steps are fused into one band-5 linear
operator whose 5 coefficient tensors are precomputed on the host:

    w_{t+2}[s] = sum_{m=0..4} C_m[s] * w_t[s-m]

Each fused block is then 5 independent tensor_tensor multiplies plus a
4-add tree on the vector engine -- 9 bf16 ops per 2 timesteps, with no
scalar-engine transcendentals anywhere. Every 8 timesteps the chain is
rescaled by a power-of-two derived from a tensor_reduce sum via a
one-instruction int32 exponent trick (bit-exactly reproducible on the
host from the exported sums, so the ln-bookkeeping happens on the host
in f64).

Fwd chain (t=0..255) and bwd chain (t=511..256, states reversed so the
shift direction matches) are packed into one [128, :] tile: partitions
0-63 fwd, 64-127 bwd. The final post-emission states + window sums are
exported and the tiny combine (one 129-wide dot per example) runs on the
host in f64. The last block's coefficients fold the final emission
instead of a trailing transition, so the export is a_255 / b'_256
directly.
"""
import numpy as np
import ml_dtypes

import concourse.bass as bass
import concourse.bacc as bacc
import concourse.mybir as mybir
from concourse import tile
from concourse.bass_utils import run_bass_kernel_spmd

B, T, C, L = 512, 512, 128, 64
S = 2 * L + 1           # 129 extended states
NCORES = 8
BS = B // NCORES        # 64 examples per core
HT = T // 2             # 256 timesteps per chain
NBLK = HT // 2          # 128 fused 2-step blocks
CW = 132                # coeff slice stride
BW = 5 * CW             # 660 cols per block in the slab
CPB = 2                 # blocks per DMA chunk (fine-grained: hides ramp)
RBLK = 4                # rescale every 4 blocks (8 timesteps)
NR = NBLK // RBLK       # 32 recorded window sums per chain
WP = 136                # state tile: 4 guards + 129 states + 3 pad
EPS = 1e-7
BLANK = C - 1
GAMMA = 0.25            # advance tilt (exact in bf16)
RK = 253 << 23          # int32 bits: r = 2^-(e+1) for ssum = m*2^e
F32 = mybir.dt.float32
I32 = mybir.dt.int32
BF16 = mybir.dt.bfloat16
ADD = mybir.AluOpType.add
SUB = mybir.AluOpType.subtract
MULT = mybir.AluOpType.mult
bf16 = ml_dtypes.bfloat16

_CACHE = {}


def _build_program():
    nc = bacc.Bacc("TRN2", target_bir_lowering=False, debug=False)
    ps = nc.dram_tensor("ps", [128, NBLK * BW], BF16, kind="ExternalInput")
    afin = nc.dram_tensor("afin", [128, WP], BF16, kind="ExternalOutput")
    ssout = nc.dram_tensor("ssums", [128, NR], F32, kind="ExternalOutput")

    with tile.TileContext(nc) as tc:
        with (
            tc.tile_pool(name="static", bufs=1) as statp,
            tc.tile_pool(name="slab", bufs=3) as slabp,
            tc.tile_pool(name="tmp", bufs=2) as tmpp,
        ):
            W = statp.tile([128, WP], BF16)
            SS = statp.tile([128, NR], F32)
            RV = statp.tile([128, NR], F32)
            nc.vector.memset(W[:, :], 0.0)
            nc.vector.memset(W[:, 4:5], 1.0)    # delta init at state 0
            nc.vector.memset(SS[:, :], 1.0)
            pending = None   # rescale whose W-scale is deferred one block
            for c in range(NBLK // CPB):
                pst = slabp.tile([128, CPB * BW], BF16, tag="ps",
                                 name="pslab")
                nc.sync.dma_start(
                    pst[:, :], ps[:, c * CPB * BW:(c + 1) * CPB * BW])
                for bi in range(CPB):
                    blk = c * CPB + bi
                    base = bi * BW
                    m = []
                    for j in range(5):
                        mj = tmpp.tile([128, S], BF16, tag=f"m{j}",
                                       name=f"m{j}")
                        nc.vector.tensor_tensor(
                            mj[:, :], W[:, 4 - j:4 - j + S],
                            pst[:, base + j * CW:base + j * CW + S], MULT)
                        m.append(mj)
                    a1 = tmpp.tile([128, S], BF16, tag="a1", name="a1")
                    nc.vector.tensor_tensor(a1[:, :], m[0][:, :],
                                            m[1][:, :], ADD)
                    a2 = tmpp.tile([128, S], BF16, tag="a2", name="a2")
                    nc.vector.tensor_tensor(a2[:, :], m[2][:, :],
                                            m[3][:, :], ADD)
                    a3 = tmpp.tile([128, S], BF16, tag="a3", name="a3")
                    nc.vector.tensor_tensor(a3[:, :], a1[:, :],
                                            a2[:, :], ADD)
                    nc.vector.tensor_tensor(W[:, 4:4 + S], a3[:, :],
                                            m[4][:, :], ADD)
                    # deferred W-scale from the previous window: applying it
                    # one block late keeps the sum+r ops fillable into this
                    # block's independent multiplies (host bookkeeping is
                    # placement-agnostic: each r applies exactly once)
                    if pending is not None:
                        nc.vector.tensor_scalar_mul(
                            W[:, 4:4 + S], W[:, 4:4 + S],
                            RV[:, pending:pending + 1])
                        pending = None
                    if blk % RBLK == RBLK - 1:
                        jj = blk // RBLK
                        nc.vector.tensor_reduce(
                            SS[:, jj:jj + 1], W[:, 4:4 + S],
                            mybir.AxisListType.X, ADD)
                        nc.vector.tensor_scalar(
                            out=RV[:, jj:jj + 1].bitcast(I32),
                            in0=SS[:, jj:jj + 1].bitcast(I32),
                            scalar1=RK, scalar2=-1, op0=SUB, op1=MULT)
                        pending = jj
            if pending is not None:
                nc.vector.tensor_scalar_mul(
                    W[:, 4:4 + S], W[:, 4:4 + S], RV[:, pending:pending + 1])
            nc.sync.dma_start(afin[:, :], W[:, :])
            nc.sync.dma_start(ssout[:, :], SS[:, :])
    nc.compile()
    return nc


def _sh(a, m):
    """Shift right along the last (state) axis by m, zero-fill."""
    if m == 0:
        return a
    return np.pad(a, [(0, 0)] * (a.ndim - 1) + [(m, 0)])[..., :a.shape[-1]]


def _host_prep(y_true, y_pred):
    yt = np.asarray(y_true)
    yp = np.asarray(y_pred, dtype=np.float32)
    ext = np.full((B, S), BLANK, np.int64)
    ext[:, 1::2] = yt
    cs = np.zeros((B, S), np.float32)
    cs[:, 2:] = ((ext[:, 2:] != BLANK)
                 & (ext[:, 2:] != ext[:, :-2])).astype(np.float32)
    p_ext = np.take_along_axis(yp, ext[:, None, :], axis=2) + np.float32(EPS)

    KB = np.zeros((B, S), np.float32)
    KB[:, 2:] = cs[:, np.arange(S - 1, 1, -1)]

    g = np.float32(GAMMA)
    PS = np.zeros((NCORES, 128, NBLK, BW), bf16)
    for ci in range(NCORES):
        ex = slice(ci * BS, (ci + 1) * BS)
        # per-row streams [128, HT, S] and masks [128, S]
        prow = np.concatenate(
            [p_ext[ex, :HT, :], p_ext[ex, :HT - 1:-1, ::-1]], axis=0)
        K = np.concatenate([cs[ex], KB[ex]], axis=0)[:, None, :]  # [128,1,S]
        p0 = prow[:, 0::2, :]     # [128, NBLK, S]
        p1 = prow[:, 1::2, :]
        Cm = np.zeros((128, NBLK, 5, S), np.float32)
        Cm[:, :, 0] = p0 * p1
        Cm[:, :, 1] = g * _sh(p0, 1) * (p1 + _sh(p1, 1))
        Cm[:, :, 2] = g * g * _sh(p0, 2) * (K * (p1 + _sh(p1, 2))
                                            + _sh(p1, 1))
        Cm[:, :, 3] = g**3 * _sh(p0, 3) * (_sh(K, 1) * _sh(p1, 1)
                                           + K * _sh(p1, 2))
        Cm[:, :, 4] = g**4 * K * _sh(K, 2) * _sh(p0, 4) * _sh(p1, 2)
        # last block: fold the final emission instead of a trailing
        # transition, so the final state is post-emission (a_255 / b'_256)
        q0, q1 = p0[:, -1, :], p1[:, -1, :]
        Cm[:, -1, 0] = q1 * q0
        Cm[:, -1, 1] = g * q1 * _sh(q0, 1)
        Cm[:, -1, 2] = g * g * K[:, 0] * q1 * _sh(q0, 2)
        Cm[:, -1, 3] = 0.0
        Cm[:, -1, 4] = 0.0
        # interleave: slice m at cols [m*CW : m*CW+S]
        view = PS[ci].reshape(128, NBLK, 5, CW)
        view[:, :, :, :S] = Cm.astype(bf16)
    return PS.reshape(NCORES, 128, NBLK * BW), cs


def _host_combine(afin, ssums, cs):
    a = afin.astype(np.float64)
    af = a[:, :BS, 4:4 + S].reshape(B, S)        # fwd final a_255
    ab = a[:, BS:, 4:4 + S].reshape(B, S)        # bwd final b'_256 (r-space)
    ssb = ssums.reshape(NCORES * 128, NR)
    r = (np.int64(RK) - ssb.view(np.int32).astype(np.int64)) \
        .astype(np.int32).view(np.float32).astype(np.float64)
    lr = np.log(r).sum(axis=1).reshape(NCORES, 128)
    laf = lr[:, :BS].reshape(B)
    lab = lr[:, BS:].reshape(B)
    g = np.float64(GAMMA)
    zg = np.zeros((B, S + 2), np.float64)
    zg[:, 2:] = af
    z = zg[:, 2:] + g * zg[:, 1:-1] + (g * g) * cs.astype(np.float64) * zg[:, 0:-2]
    dot = (z * ab[:, ::-1]).sum(axis=1)
    # stored chains carry factor prod(r); ln true = ln stored - sum ln r
    ll = (np.log(np.maximum(dot, 1e-300)) - laf - lab
          - (S - 1) * np.log(g))
    return (-ll[:, None]).astype(np.float32)


def kernel(y_true, y_pred):
    PS, cs = _host_prep(y_true, y_pred)
    if "nc" not in _CACHE:
        _CACHE["nc"] = _build_program()
    nc = _CACHE["nc"]
    in_maps = [{"ps": PS[i]} for i in range(NCORES)]
    res = run_bass_kernel_spmd(nc, in_maps, core_ids=list(range(NCORES)))
    afin = np.stack([res.results[i]["afin"] for i in range(NCORES)])
    ssums = np.stack([res.results[i]["ssums"] for i in range(NCORES)])
    return _host_combine(afin, ssums, cs)



# revision 2
# speedup vs baseline: 2.1717x; 2.1717x over previous
"""CTC loss (keras ctc_batch_cost semantics) on 8 Trainium2 NeuronCores.

Parity-normalized scan formulation. The CTC extended-state DP alternates
blank (even) and label (odd) states. All even states emit the same blank
probability pB_t, so normalizing the whole state vector by the running
blank product turns every even-state update into a pure shift-add and
every odd-state update into an affine recurrence in the emission RATIO
q_t = pl_t/pB_t. Each state-pair then reduces to three DVE instructions
over the full 256-step time axis:

    scanE:  E_t = delta_t * (E_{t-1} + O[j-1]_{t-1})     (cumsum-scan)
    stt:    b_t = K'_j * O[j-1]_{t-1} + E[j]_{t-1}       (fused mul-add)
    scanO:  O_t = qt[j]_t * (O_{t-1} + b_t)              (affine scan)

where qt = q * delta folds a per-(example,t) damping series delta chosen
from a mean-field surrogate so stored magnitudes stay O(1), and a
per-example tilt r applied every TILT_EVERY pairs (one tensor_scalar plus
host-folded K' = K*r) flattens the exponential state profile so bf16
storage holds the junction products. Forward (t<256) and reverse
(t>=256, states reversed) chains for 64 examples pack the 128 partitions
of each core; the final columns are gathered with two strided copies and
combined on the host in f64 with exact log-corrections for the blank
product, damping, and tilt ledgers.
"""
import numpy as np
import ml_dtypes

import concourse.bass as bass
import concourse.bacc as bacc
import concourse.mybir as mybir
from concourse import tile
from concourse.bass_utils import run_bass_kernel_spmd

B, T, C, L = 512, 512, 128, 64
S = 2 * L + 1
NCORES = 8
BS = B // NCORES        # 64 examples per core
HT = T // 2             # 256 timesteps per chain
BLANK = C - 1
EPS = 1e-7
TILT_EVERY = 4
PW = HT + 1             # per-pair series stride (1 guard col + 256)
NQCHUNK = 8             # QS input DMA chunks
CF0, CF1 = 1.5689666, 2.17334313   # junction profile slope vs mean ln q

F32 = mybir.dt.float32
BF16 = mybir.dt.bfloat16
ADD = mybir.AluOpType.add
MULT = mybir.AluOpType.mult
bf16 = ml_dtypes.bfloat16

_CACHE = {}


def _build_program():
    nc = bacc.Bacc("TRN2", target_bir_lowering=False, debug=False)
    qs = nc.dram_tensor("qs", [128, L * HT], BF16, kind="ExternalInput")
    dl = nc.dram_tensor("dl", [128, HT], BF16, kind="ExternalInput")
    kc = nc.dram_tensor("kc", [128, L], BF16, kind="ExternalInput")
    gr = nc.dram_tensor("gr", [128, 1], F32, kind="ExternalInput")
    afin = nc.dram_tensor("afin", [128, 132], BF16, kind="ExternalOutput")

    with tile.TileContext(nc) as tc:
        with tc.tile_pool(name="static", bufs=1) as sp:
            ES = sp.tile([128, (L + 1) * PW], BF16)
            OS = sp.tile([128, L * PW], BF16)
            QS = sp.tile([128, L * HT], BF16)
            ZT = sp.tile([128, HT], BF16)
            DL = sp.tile([128, HT], BF16)
            KC = sp.tile([128, L], BF16)
            GR = sp.tile([128, 1], F32)
            GT = [sp.tile([128, HT], BF16, name=f"gt{i}") for i in range(2)]
            BT = [sp.tile([128, HT], BF16, name=f"bt{i}") for i in range(2)]
            EX = sp.tile([128, 132], BF16)

            nc.sync.dma_start(DL[:, :], dl[:, :])
            nc.sync.dma_start(KC[:, :], kc[:, :])
            nc.sync.dma_start(GR[:, :], gr[:, :])
            ck = (L * HT) // NQCHUNK
            for c in range(NQCHUNK):
                nc.sync.dma_start(QS[:, c * ck:(c + 1) * ck],
                                  qs[:, c * ck:(c + 1) * ck])
            nc.vector.memset(ZT[:, :], 0.0)
            # guard columns: E_{-1}[j] = [j == 0], O_{-1}[j] = 0
            nc.vector.memset(ES[:, 0:(L + 1) * PW:PW], 0.0)
            nc.vector.memset(OS[:, 0:L * PW:PW], 0.0)
            nc.vector.memset(ES[:, 0:1], 1.0)
            nc.vector.memset(EX[:, :], 0.0)

            for j in range(L + 1):
                ob = (j - 1) * PW
                osh = ZT[:, 0:HT] if j == 0 else OS[:, ob:ob + HT]
                tilted = (j > 0) and (j % TILT_EVERY == 0)
                if tilted:
                    g = GT[(j // TILT_EVERY) % 2]
                    nc.vector.tensor_scalar_mul(g[:, :], osh, GR[:, 0:1])
                    d0e = g[:, 0:HT]
                else:
                    d0e = osh
                eb = j * PW
                nc.vector.tensor_tensor_scan(
                    ES[:, eb + 1:eb + 1 + HT], d0e, DL[:, 0:HT],
                    1.0 if j == 0 else 0.0, ADD, MULT)
                if j == L:
                    break
                b = BT[j % 2]
                nc.vector.scalar_tensor_tensor(
                    b[:, :], osh, KC[:, j:j + 1], ES[:, eb:eb + HT],
                    MULT, ADD)
                nc.vector.tensor_tensor_scan(
                    OS[:, ob + PW + 1:ob + PW + 1 + HT], b[:, 0:HT],
                    QS[:, j * HT:(j + 1) * HT], 0.0, ADD, MULT)

            nc.vector.tensor_copy(EX[:, 0:L + 1], ES[:, HT::PW])
            nc.vector.tensor_copy(EX[:, L + 1:S], OS[:, HT::PW])
            nc.sync.dma_start(afin[:, :], EX[:, :])
    nc.compile()
    return nc


def _host_prep(y_true, y_pred):
    yt = np.asarray(y_true)
    yp = np.asarray(y_pred, dtype=np.float32)
    pB = yp[:, :, BLANK].astype(np.float64) + EPS            # [B, T]
    pl = (np.take_along_axis(yp, yt[:, None, :].astype(np.int64), axis=2)
          .astype(np.float64) + EPS)                          # [B, T, L]

    # fwd chain (t < HT) and bwd chain (reversed time + labels)
    q_f = pl[:, :HT, :] / pB[:, :HT, None]
    q_b = pl[:, :HT - 1:-1, ::-1] / pB[:, :HT - 1:-1, None]
    K_f = np.zeros((B, L))
    K_f[:, 1:] = (yt[:, 1:] != yt[:, :-1]).astype(np.float64)
    K_b = np.zeros((B, L))
    K_b[:, 1:] = (yt[:, ::-1][:, 1:] != yt[:, ::-1][:, :-1]).astype(np.float64)

    def chain_params(q):
        lnq = np.log(q).mean(axis=(1, 2))
        slope = CF0 * lnq + CF1
        gam = np.exp(-slope)
        r = gam ** (2 * TILT_EVERY)
        # damping from tilted 2-state mean-field surrogate
        e = np.ones((B,)); o = np.zeros((B,))
        g = np.empty((B, HT))
        qb = q.mean(axis=2)
        for t in range(HT):
            e2 = e + gam * o
            o2 = qb[:, t] * (o + gam * e + gam * gam * o)
            z2 = e2 + o2
            g[:, t] = z2 / (e + o)
            e, o = e2 / z2, o2 / z2
        delta = 1.0 / g
        return r, delta

    r_f, d_f = chain_params(q_f)
    r_b, d_b = chain_params(q_b)

    def pack(q, K, r, delta):
        # qs rows: [n, L*HT] pair-major; kc with K' = K*r on tilted hops
        qt = (q * delta[:, :, None]).transpose(0, 2, 1)       # [n, L, HT]
        qs = qt.reshape(q.shape[0], L * HT).astype(bf16)
        kc = K.copy()
        for j in range(TILT_EVERY, L, TILT_EVERY):
            kc[:, j] *= r
        return qs, kc.astype(bf16), delta.astype(bf16), r.astype(np.float32)

    qs_f, kc_f, dl_f, gr_f = pack(q_f, K_f, r_f, d_f)
    qs_b, kc_b, dl_b, gr_b = pack(q_b, K_b, r_b, d_b)

    in_maps = []
    for ci in range(NCORES):
        ex = slice(ci * BS, (ci + 1) * BS)
        in_maps.append({
            "qs": np.concatenate([qs_f[ex], qs_b[ex]], axis=0),
            "dl": np.concatenate([dl_f[ex], dl_b[ex]], axis=0),
            "kc": np.concatenate([kc_f[ex], kc_b[ex]], axis=0),
            "gr": np.concatenate([gr_f[ex], gr_b[ex]], axis=0)[:, None],
        })
    aux = (pB, r_f, r_b, d_f, d_b, yt)
    return in_maps, aux


def _host_combine(afin, aux):
    pB, r_f, r_b, d_f, d_b, yt = aux
    af_s = afin[:, :BS, :].reshape(B, 132).astype(np.float64)
    ab_s = afin[:, BS:, :].reshape(B, 132).astype(np.float64)

    # un-tilt ledger: pair j carries floor(j / TILT_EVERY) factors of r
    nt = np.floor_divide(np.arange(L + 1), TILT_EVERY)
    af = np.zeros((B, S)); ab = np.zeros((B, S))
    af[:, 0::2] = af_s[:, 0:L + 1] * r_f[:, None] ** (-nt[None, :])
    af[:, 1::2] = af_s[:, L + 1:S] * r_f[:, None] ** (-nt[None, :L])
    ab[:, 0::2] = ab_s[:, 0:L + 1] * r_b[:, None] ** (-nt[None, :])
    ab[:, 1::2] = ab_s[:, L + 1:S] * r_b[:, None] ** (-nt[None, :L])

    ext = np.full((B, S), BLANK, np.int64)
    ext[:, 1::2] = yt
    cs = np.zeros((B, S))
    cs[:, 2:] = ((ext[:, 2:] != BLANK)
                 & (ext[:, 2:] != ext[:, :-2])).astype(np.float64)
    zg = np.zeros((B, S + 2))
    zg[:, 2:] = af
    z = zg[:, 2:] + zg[:, 1:-1] + cs * zg[:, 0:-2]
    dot = (z * ab[:, ::-1]).sum(axis=1)

    lnF = np.log(pB).sum(axis=1)
    lnD = np.log(d_f).sum(axis=1) + np.log(d_b).sum(axis=1)
    ll = np.log(np.maximum(dot, 1e-300)) + lnF - lnD
    return (-ll[:, None]).astype(np.float32)


def kernel(y_true, y_pred):
    in_maps, aux = _host_prep(y_true, y_pred)
    if "nc" not in _CACHE:
        _CACHE["nc"] = _build_program()
    nc = _CACHE["nc"]
    res = run_bass_kernel_spmd(nc, in_maps, core_ids=list(range(NCORES)))
    afin = np.stack([np.asarray(res.results[i]["afin"], dtype=np.float32)
                     for i in range(NCORES)])
    return _host_combine(afin, aux)


# revision 6
# speedup vs baseline: 2.2409x; 1.0318x over previous
"""CTC loss (keras ctc_batch_cost semantics) on 8 Trainium2 NeuronCores.

Parity-normalized scan formulation. The CTC extended-state DP alternates
blank (even) and label (odd) states. All even states emit the same blank
probability pB_t, so normalizing the whole state vector by the running
blank product turns every even-state update into a pure shift-add and
every odd-state update into an affine recurrence in the emission RATIO
q_t = pl_t/pB_t. Each state-pair then reduces to three DVE instructions
over the full 256-step time axis:

    scanE:  E_t = delta_t * (E_{t-1} + O[j-1]_{t-1})     (cumsum-scan)
    stt:    b_t = K'_j * O[j-1]_{t-1} + E[j]_{t-1}       (fused mul-add)
    scanO:  O_t = qt[j]_t * (O_{t-1} + b_t)              (affine scan)

where qt = q * delta folds a per-(example,t) damping series delta chosen
from a mean-field surrogate so stored magnitudes stay O(1), and a
per-example tilt r applied every TILT_EVERY pairs (one tensor_scalar plus
host-folded K' = K*r) flattens the exponential state profile so bf16
storage holds the junction products. Forward (t<256) and reverse
(t>=256, states reversed) chains for 64 examples pack the 128 partitions
of each core; the final columns are gathered with two strided copies and
combined on the host in f64 with exact log-corrections for the blank
product, damping, and tilt ledgers.
"""
import numpy as np
import ml_dtypes

import concourse.bass as bass
import concourse.bacc as bacc
import concourse.mybir as mybir
from concourse import tile
from concourse.bass_utils import run_bass_kernel_spmd

B, T, C, L = 512, 512, 128, 64
S = 2 * L + 1
NCORES = 8
BS = B // NCORES        # 64 examples per core
HT = T // 2             # 256 timesteps per chain
BLANK = C - 1
EPS = 1e-7
TILT_EVERY = 8
PW = HT + 1             # per-pair series stride (1 guard col + 256)
QCHUNKS = (2, 6, 8, 8, 8, 8, 8, 8, 8)   # QS DMA chunk sizes in pairs
CF0, CF1 = 1.5689666, 2.17334313   # junction profile slope vs mean ln q

F32 = mybir.dt.float32
BF16 = mybir.dt.bfloat16
ADD = mybir.AluOpType.add
MULT = mybir.AluOpType.mult
bf16 = ml_dtypes.bfloat16

_CACHE = {}


def _build_program():
    nc = bacc.Bacc("TRN2", target_bir_lowering=False, debug=False)
    qs = nc.dram_tensor("qs", [128, L * HT], BF16, kind="ExternalInput")
    dl = nc.dram_tensor("dl", [128, HT], BF16, kind="ExternalInput")
    kc = nc.dram_tensor("kc", [128, L], BF16, kind="ExternalInput")
    gr = nc.dram_tensor("gr", [128, 1], F32, kind="ExternalInput")
    afin = nc.dram_tensor("afin", [128, 132], BF16, kind="ExternalOutput")

    with tile.TileContext(nc) as tc:
        with tc.tile_pool(name="static", bufs=1) as sp:
            ES = sp.tile([128, (L + 1) * PW], BF16)
            OS = sp.tile([128, L * PW], BF16)
            QS = sp.tile([128, L * HT], BF16)
            ZT = sp.tile([128, HT], BF16)
            DL = sp.tile([128, HT], BF16)
            KC = sp.tile([128, L], BF16)
            GR = sp.tile([128, 1], F32)
            GT = [sp.tile([128, HT], BF16, name=f"gt{i}") for i in range(2)]
            BT = [sp.tile([128, HT], BF16, name=f"bt{i}") for i in range(2)]
            EX = sp.tile([128, 132], BF16)

            nc.sync.dma_start(DL[:, :], dl[:, :])
            nc.sync.dma_start(KC[:, :], kc[:, :])
            nc.sync.dma_start(GR[:, :], gr[:, :])
            pos = 0
            for npair in QCHUNKS:
                a, b = pos * HT, (pos + npair) * HT
                nc.sync.dma_start(QS[:, a:b], qs[:, a:b])
                pos += npair
            nc.vector.memset(ZT[:, :], 0.0)
            # guard columns: E_{-1}[j] = [j == 0], O_{-1}[j] = 0
            nc.vector.memset(ES[:, 0:(L + 1) * PW:PW], 0.0)
            nc.vector.memset(OS[:, 0:L * PW:PW], 0.0)
            nc.vector.memset(ES[:, 0:1], 1.0)
            nc.vector.memset(EX[:, :], 0.0)

            for j in range(L + 1):
                ob = (j - 1) * PW
                osh = ZT[:, 0:HT] if j == 0 else OS[:, ob:ob + HT]
                tilted = (j > 0) and (j % TILT_EVERY == 0)
                if tilted:
                    g = GT[(j // TILT_EVERY) % 2]
                    nc.vector.tensor_scalar_mul(g[:, :], osh, GR[:, 0:1])
                    d0e = g[:, 0:HT]
                else:
                    d0e = osh
                eb = j * PW
                nc.vector.tensor_tensor_scan(
                    ES[:, eb + 1:eb + 1 + HT], d0e, DL[:, 0:HT],
                    1.0 if j == 0 else 0.0, ADD, MULT)
                if j == L:
                    break
                b = BT[j % 2]
                nc.vector.scalar_tensor_tensor(
                    b[:, :], osh, KC[:, j:j + 1], ES[:, eb:eb + HT],
                    MULT, ADD)
                nc.vector.tensor_tensor_scan(
                    OS[:, ob + PW + 1:ob + PW + 1 + HT], b[:, 0:HT],
                    QS[:, j * HT:(j + 1) * HT], 0.0, ADD, MULT)
                if j == 47:
                    # early export of finished pairs; hides copy latency
                    nc.vector.tensor_copy(EX[:, 0:48], ES[:, HT:48 * PW:PW])
                    nc.vector.tensor_copy(EX[:, L + 1:L + 48],
                                          OS[:, HT:47 * PW:PW])

            nc.vector.tensor_copy(EX[:, 48:L + 1], ES[:, 48 * PW + HT::PW])
            nc.vector.tensor_copy(EX[:, L + 48:S], OS[:, 47 * PW + HT::PW])
            nc.sync.dma_start(afin[:, :], EX[:, :])
    nc.compile()
    return nc


def _host_prep(y_true, y_pred):
    yt = np.asarray(y_true)
    yp = np.asarray(y_pred, dtype=np.float32)
    pB = yp[:, :, BLANK].astype(np.float64) + EPS            # [B, T]
    pl = (np.take_along_axis(yp, yt[:, None, :].astype(np.int64), axis=2)
          .astype(np.float64) + EPS)                          # [B, T, L]

    # fwd chain (t < HT) and bwd chain (reversed time + labels)
    q_f = pl[:, :HT, :] / pB[:, :HT, None]
    q_b = pl[:, :HT - 1:-1, ::-1] / pB[:, :HT - 1:-1, None]
    K_f = np.zeros((B, L))
    K_f[:, 1:] = (yt[:, 1:] != yt[:, :-1]).astype(np.float64)
    K_b = np.zeros((B, L))
    K_b[:, 1:] = (yt[:, ::-1][:, 1:] != yt[:, ::-1][:, :-1]).astype(np.float64)

    def chain_params(q):
        lnq = np.log(q).mean(axis=(1, 2))
        slope = CF0 * lnq + CF1
        gam = np.exp(-slope)
        r = gam ** (2 * TILT_EVERY)
        # damping from tilted 2-state mean-field surrogate
        e = np.ones((B,)); o = np.zeros((B,))
        g = np.empty((B, HT))
        qb = q.mean(axis=2)
        for t in range(HT):
            e2 = e + gam * o
            o2 = qb[:, t] * (o + gam * e + gam * gam * o)
            z2 = e2 + o2
            g[:, t] = z2 / (e + o)
            e, o = e2 / z2, o2 / z2
        delta = 1.0 / g
        return r, delta

    r_f, d_f = chain_params(q_f)
    r_b, d_b = chain_params(q_b)

    def pack(q, K, r, delta):
        # qs rows: [n, L*HT] pair-major; kc with K' = K*r on tilted hops
        qt = (q * delta[:, :, None]).transpose(0, 2, 1)       # [n, L, HT]
        qs = qt.reshape(q.shape[0], L * HT).astype(bf16)
        kc = K.copy()
        for j in range(TILT_EVERY, L, TILT_EVERY):
            kc[:, j] *= r
        return qs, kc.astype(bf16), delta.astype(bf16), r.astype(np.float32)

    qs_f, kc_f, dl_f, gr_f = pack(q_f, K_f, r_f, d_f)
    qs_b, kc_b, dl_b, gr_b = pack(q_b, K_b, r_b, d_b)

    in_maps = []
    for ci in range(NCORES):
        ex = slice(ci * BS, (ci + 1) * BS)
        in_maps.append({
            "qs": np.concatenate([qs_f[ex], qs_b[ex]], axis=0),
            "dl": np.concatenate([dl_f[ex], dl_b[ex]], axis=0),
            "kc": np.concatenate([kc_f[ex], kc_b[ex]], axis=0),
            "gr": np.concatenate([gr_f[ex], gr_b[ex]], axis=0)[:, None],
        })
    aux = (pB, r_f, r_b, d_f, d_b, yt)
    return in_maps, aux


def _host_combine(afin, aux):
    pB, r_f, r_b, d_f, d_b, yt = aux
    af_s = afin[:, :BS, :].reshape(B, 132).astype(np.float64)
    ab_s = afin[:, BS:, :].reshape(B, 132).astype(np.float64)

    # un-tilt ledger: pair j carries floor(j / TILT_EVERY) factors of r
    nt = np.floor_divide(np.arange(L + 1), TILT_EVERY)
    af = np.zeros((B, S)); ab = np.zeros((B, S))
    af[:, 0::2] = af_s[:, 0:L + 1] * r_f[:, None] ** (-nt[None, :])
    af[:, 1::2] = af_s[:, L + 1:S] * r_f[:, None] ** (-nt[None, :L])
    ab[:, 0::2] = ab_s[:, 0:L + 1] * r_b[:, None] ** (-nt[None, :])
    ab[:, 1::2] = ab_s[:, L + 1:S] * r_b[:, None] ** (-nt[None, :L])

    ext = np.full((B, S), BLANK, np.int64)
    ext[:, 1::2] = yt
    cs = np.zeros((B, S))
    cs[:, 2:] = ((ext[:, 2:] != BLANK)
                 & (ext[:, 2:] != ext[:, :-2])).astype(np.float64)
    zg = np.zeros((B, S + 2))
    zg[:, 2:] = af
    z = zg[:, 2:] + zg[:, 1:-1] + cs * zg[:, 0:-2]
    dot = (z * ab[:, ::-1]).sum(axis=1)

    lnF = np.log(pB).sum(axis=1)
    lnD = np.log(d_f).sum(axis=1) + np.log(d_b).sum(axis=1)
    ll = np.log(np.maximum(dot, 1e-300)) + lnF - lnD
    return (-ll[:, None]).astype(np.float32)


def kernel(y_true, y_pred):
    in_maps, aux = _host_prep(y_true, y_pred)
    if "nc" not in _CACHE:
        _CACHE["nc"] = _build_program()
    nc = _CACHE["nc"]
    res = run_bass_kernel_spmd(nc, in_maps, core_ids=list(range(NCORES)))
    afin = np.stack([np.asarray(res.results[i]["afin"], dtype=np.float32)
                     for i in range(NCORES)])
    return _host_combine(afin, aux)


# revision 9
# speedup vs baseline: 2.2665x; 1.0114x over previous
"""CTC loss (keras ctc_batch_cost semantics) on 8 Trainium2 NeuronCores.

Parity-normalized scan formulation. The CTC extended-state DP alternates
blank (even) and label (odd) states. All even states emit the same blank
probability pB_t, so normalizing the whole state vector by the running
blank product turns every even-state update into a pure shift-add and
every odd-state update into an affine recurrence in the emission RATIO
q_t = pl_t/pB_t. Each state-pair then reduces to three DVE instructions
over the full 256-step time axis:

    scanE:  E_t = delta_t * (E_{t-1} + O[j-1]_{t-1})     (cumsum-scan)
    stt:    b_t = K'_j * O[j-1]_{t-1} + E[j]_{t-1}       (fused mul-add)
    scanO:  O_t = qt[j]_t * (O_{t-1} + b_t)              (affine scan)

where qt = q * delta folds a per-(example,t) damping series delta chosen
from a mean-field surrogate so stored magnitudes stay O(1), and a
per-example tilt r applied every TILT_EVERY pairs (one tensor_scalar plus
host-folded K' = K*r) flattens the exponential state profile so bf16
storage holds the junction products. Forward (t<256) and reverse
(t>=256, states reversed) chains for 64 examples pack the 128 partitions
of each core; the final columns are gathered with two strided copies and
combined on the host in f64 with exact log-corrections for the blank
product, damping, and tilt ledgers.
"""
import numpy as np
import ml_dtypes

import concourse.bass as bass
import concourse.bacc as bacc
import concourse.mybir as mybir
from concourse import tile
from concourse.bass_utils import run_bass_kernel_spmd

B, T, C, L = 512, 512, 128, 64
S = 2 * L + 1
NCORES = 8
BS = B // NCORES        # 64 examples per core
HT = T // 2             # 256 timesteps per chain
BLANK = C - 1
EPS = 1e-7
TILT_EVERY = 8
PW = HT + 1             # per-pair series stride (1 guard col + 256)
QCHUNKS = (2, 6, 8, 8, 8, 8, 8, 8, 8)   # QS DMA chunk sizes in pairs
CF0, CF1 = 1.5689666, 2.17334313   # junction profile slope vs mean ln q

F32 = mybir.dt.float32
BF16 = mybir.dt.bfloat16
ADD = mybir.AluOpType.add
MULT = mybir.AluOpType.mult
bf16 = ml_dtypes.bfloat16

_CACHE = {}


HDR = HT + L            # header cols: dl series + kc columns


def _build_program():
    nc = bacc.Bacc("TRN2", target_bir_lowering=False, debug=False)
    qs = nc.dram_tensor("qs", [128, HDR + L * HT], BF16, kind="ExternalInput")
    gr = nc.dram_tensor("gr", [128, 1], F32, kind="ExternalInput")
    afin = nc.dram_tensor("afin", [128, 132], BF16, kind="ExternalOutput")

    with tile.TileContext(nc) as tc:
        with tc.tile_pool(name="static", bufs=1) as sp:
            ES = sp.tile([128, (L + 1) * PW], BF16)
            OS = sp.tile([128, L * PW], BF16)
            QS = sp.tile([128, HDR + L * HT], BF16)
            ZT = sp.tile([128, HT], BF16)
            GR = sp.tile([128, 1], F32)
            GT = [sp.tile([128, HT], BF16, name=f"gt{i}") for i in range(2)]
            BT = [sp.tile([128, HT], BF16, name=f"bt{i}") for i in range(2)]
            EX = sp.tile([128, 132], BF16)
            # header + first pairs land in chunk 0; gr is not needed until
            # the first tilt hop (pair 8), so its DMA rides later
            pos = 0
            for i, npair in enumerate(QCHUNKS):
                a = 0 if i == 0 else HDR + pos * HT
                b = HDR + (pos + npair) * HT
                nc.sync.dma_start(QS[:, a:b], qs[:, a:b])
                pos += npair
                if i == 1:
                    nc.sync.dma_start(GR[:, :], gr[:, :])
            nc.vector.memset(ZT[:, :], 0.0)
            # guard columns: E_{-1}[j] = [j == 0], O_{-1}[j] = 0
            nc.vector.memset(ES[:, 0:(L + 1) * PW:PW], 0.0)
            nc.vector.memset(OS[:, 0:L * PW:PW], 0.0)
            nc.vector.memset(ES[:, 0:1], 1.0)
            nc.vector.memset(EX[:, 129:132], 0.0)

            for j in range(L + 1):
                ob = (j - 1) * PW
                osh = ZT[:, 0:HT] if j == 0 else OS[:, ob:ob + HT]
                tilted = (j > 0) and (j % TILT_EVERY == 0)
                if tilted:
                    g = GT[(j // TILT_EVERY) % 2]
                    nc.vector.tensor_scalar_mul(g[:, :], osh, GR[:, 0:1])
                    d0e = g[:, 0:HT]
                else:
                    d0e = osh
                eb = j * PW
                nc.vector.tensor_tensor_scan(
                    ES[:, eb + 1:eb + 1 + HT], d0e, QS[:, 0:HT],
                    1.0 if j == 0 else 0.0, ADD, MULT)
                if j == L:
                    break
                b = BT[j % 2]
                nc.vector.scalar_tensor_tensor(
                    b[:, :], osh, QS[:, HT + j:HT + j + 1], ES[:, eb:eb + HT],
                    MULT, ADD)
                nc.vector.tensor_tensor_scan(
                    OS[:, ob + PW + 1:ob + PW + 1 + HT], b[:, 0:HT],
                    QS[:, HDR + j * HT:HDR + (j + 1) * HT], 0.0, ADD, MULT)
                if j == 47:
                    # early export of finished pairs; hides copy latency
                    nc.vector.tensor_copy(EX[:, 0:48], ES[:, HT:48 * PW:PW])
                    nc.vector.tensor_copy(EX[:, L + 1:L + 48],
                                          OS[:, HT:47 * PW:PW])

            nc.vector.tensor_copy(EX[:, 48:L + 1], ES[:, 48 * PW + HT::PW])
            nc.vector.tensor_copy(EX[:, L + 48:S], OS[:, 47 * PW + HT::PW])
            nc.sync.dma_start(afin[:, :], EX[:, :])
    nc.compile()
    return nc


def _host_prep(y_true, y_pred):
    yt = np.asarray(y_true)
    yp = np.asarray(y_pred, dtype=np.float32)
    pB = yp[:, :, BLANK].astype(np.float64) + EPS            # [B, T]
    pl = (np.take_along_axis(yp, yt[:, None, :].astype(np.int64), axis=2)
          .astype(np.float64) + EPS)                          # [B, T, L]

    # fwd chain (t < HT) and bwd chain (reversed time + labels)
    q_f = pl[:, :HT, :] / pB[:, :HT, None]
    q_b = pl[:, :HT - 1:-1, ::-1] / pB[:, :HT - 1:-1, None]
    K_f = np.zeros((B, L))
    K_f[:, 1:] = (yt[:, 1:] != yt[:, :-1]).astype(np.float64)
    K_b = np.zeros((B, L))
    K_b[:, 1:] = (yt[:, ::-1][:, 1:] != yt[:, ::-1][:, :-1]).astype(np.float64)

    def chain_params(q):
        lnq = np.log(q).mean(axis=(1, 2))
        slope = CF0 * lnq + CF1
        gam = np.exp(-slope)
        r = gam ** (2 * TILT_EVERY)
        # damping from tilted 2-state mean-field surrogate
        e = np.ones((B,)); o = np.zeros((B,))
        g = np.empty((B, HT))
        qb = q.mean(axis=2)
        for t in range(HT):
            e2 = e + gam * o
            o2 = qb[:, t] * (o + gam * e + gam * gam * o)
            z2 = e2 + o2
            g[:, t] = z2 / (e + o)
            e, o = e2 / z2, o2 / z2
        delta = 1.0 / g
        return r, delta

    r_f, d_f = chain_params(q_f)
    r_b, d_b = chain_params(q_b)

    def pack(q, K, r, delta):
        # qs rows: [dl series | K' columns | pair-major q*delta series]
        n = q.shape[0]
        qt = (q * delta[:, :, None]).transpose(0, 2, 1)       # [n, L, HT]
        kc = K.copy()
        for j in range(TILT_EVERY, L, TILT_EVERY):
            kc[:, j] *= r
        qs = np.concatenate(
            [delta, kc, qt.reshape(n, L * HT)], axis=1).astype(bf16)
        return qs, r.astype(np.float32)

    qs_f, gr_f = pack(q_f, K_f, r_f, d_f)
    qs_b, gr_b = pack(q_b, K_b, r_b, d_b)

    in_maps = []
    for ci in range(NCORES):
        ex = slice(ci * BS, (ci + 1) * BS)
        in_maps.append({
            "qs": np.concatenate([qs_f[ex], qs_b[ex]], axis=0),
            "gr": np.concatenate([gr_f[ex], gr_b[ex]], axis=0)[:, None],
        })
    aux = (pB, r_f, r_b, d_f, d_b, yt)
    return in_maps, aux


def _host_combine(afin, aux):
    pB, r_f, r_b, d_f, d_b, yt = aux
    af_s = afin[:, :BS, :].reshape(B, 132).astype(np.float64)
    ab_s = afin[:, BS:, :].reshape(B, 132).astype(np.float64)

    # un-tilt ledger: pair j carries floor(j / TILT_EVERY) factors of r
    nt = np.floor_divide(np.arange(L + 1), TILT_EVERY)
    af = np.zeros((B, S)); ab = np.zeros((B, S))
    af[:, 0::2] = af_s[:, 0:L + 1] * r_f[:, None] ** (-nt[None, :])
    af[:, 1::2] = af_s[:, L + 1:S] * r_f[:, None] ** (-nt[None, :L])
    ab[:, 0::2] = ab_s[:, 0:L + 1] * r_b[:, None] ** (-nt[None, :])
    ab[:, 1::2] = ab_s[:, L + 1:S] * r_b[:, None] ** (-nt[None, :L])

    ext = np.full((B, S), BLANK, np.int64)
    ext[:, 1::2] = yt
    cs = np.zeros((B, S))
    cs[:, 2:] = ((ext[:, 2:] != BLANK)
                 & (ext[:, 2:] != ext[:, :-2])).astype(np.float64)
    zg = np.zeros((B, S + 2))
    zg[:, 2:] = af
    z = zg[:, 2:] + zg[:, 1:-1] + cs * zg[:, 0:-2]
    dot = (z * ab[:, ::-1]).sum(axis=1)

    lnF = np.log(pB).sum(axis=1)
    lnD = np.log(d_f).sum(axis=1) + np.log(d_b).sum(axis=1)
    ll = np.log(np.maximum(dot, 1e-300)) + lnF - lnD
    return (-ll[:, None]).astype(np.float32)


def kernel(y_true, y_pred):
    in_maps, aux = _host_prep(y_true, y_pred)
    if "nc" not in _CACHE:
        _CACHE["nc"] = _build_program()
    nc = _CACHE["nc"]
    res = run_bass_kernel_spmd(nc, in_maps, core_ids=list(range(NCORES)))
    afin = np.stack([np.asarray(res.results[i]["afin"], dtype=np.float32)
                     for i in range(NCORES)])
    return _host_combine(afin, aux)


# revision 10
# speedup vs baseline: 2.2670x; 1.0002x over previous
"""CTC loss (keras ctc_batch_cost semantics) on 8 Trainium2 NeuronCores.

Parity-normalized scan formulation. The CTC extended-state DP alternates
blank (even) and label (odd) states. All even states emit the same blank
probability pB_t, so normalizing the whole state vector by the running
blank product turns every even-state update into a pure shift-add and
every odd-state update into an affine recurrence in the emission RATIO
q_t = pl_t/pB_t. Each state-pair then reduces to three DVE instructions
over the full 256-step time axis:

    scanE:  E_t = delta_t * (E_{t-1} + O[j-1]_{t-1})     (cumsum-scan)
    stt:    b_t = K'_j * O[j-1]_{t-1} + E[j]_{t-1}       (fused mul-add)
    scanO:  O_t = qt[j]_t * (O_{t-1} + b_t)              (affine scan)

where qt = q * delta folds a per-(example,t) damping series delta chosen
from a mean-field surrogate so stored magnitudes stay O(1), and a
per-example tilt r applied every TILT_EVERY pairs (one tensor_scalar plus
host-folded K' = K*r) flattens the exponential state profile so bf16
storage holds the junction products. Forward (t<256) and reverse
(t>=256, states reversed) chains for 64 examples pack the 128 partitions
of each core; the final columns are gathered with two strided copies and
combined on the host in f64 with exact log-corrections for the blank
product, damping, and tilt ledgers.
"""
import numpy as np
import ml_dtypes

import concourse.bass as bass
import concourse.bacc as bacc
import concourse.mybir as mybir
from concourse import tile
from concourse.bass_utils import run_bass_kernel_spmd

B, T, C, L = 512, 512, 128, 64
S = 2 * L + 1
NCORES = 8
BS = B // NCORES        # 64 examples per core
HT = T // 2             # 256 timesteps per chain
BLANK = C - 1
EPS = 1e-7
TILT_EVERY = 8
PW = HT + 1             # per-pair series stride (1 guard col + 256)
QCHUNKS = (2, 6, 8, 8, 8, 8, 8, 8, 8)   # QS DMA chunk sizes in pairs
CF0, CF1 = 1.5689666, 2.17334313   # junction profile slope vs mean ln q

F32 = mybir.dt.float32
BF16 = mybir.dt.bfloat16
ADD = mybir.AluOpType.add
MULT = mybir.AluOpType.mult
bf16 = ml_dtypes.bfloat16

_CACHE = {}


HDR = HT + L            # header cols: dl series + kc columns


def _build_program():
    nc = bacc.Bacc("TRN2", target_bir_lowering=False, debug=False)
    qs = nc.dram_tensor("qs", [128, HDR + L * HT], BF16, kind="ExternalInput")
    gr = nc.dram_tensor("gr", [128, 1], F32, kind="ExternalInput")
    afin = nc.dram_tensor("afin", [128, 132], BF16, kind="ExternalOutput")

    with tile.TileContext(nc) as tc:
        with tc.tile_pool(name="static", bufs=1) as sp:
            ES = sp.tile([128, (L + 1) * PW], BF16)
            OS = sp.tile([128, L * PW], BF16)
            QS = sp.tile([128, HDR + L * HT], BF16)
            ZT = sp.tile([128, HT], BF16)
            GR = sp.tile([128, 1], F32)
            GT = [sp.tile([128, HT], BF16, name=f"gt{i}") for i in range(2)]
            BT = [sp.tile([128, HT], BF16, name=f"bt{i}") for i in range(2)]
            EX = sp.tile([128, 132], BF16)
            # header + first pairs land in chunk 0; gr is not needed until
            # the first tilt hop (pair 8), so its DMA rides later
            pos = 0
            for i, npair in enumerate(QCHUNKS):
                a = 0 if i == 0 else HDR + pos * HT
                b = HDR + (pos + npair) * HT
                nc.sync.dma_start(QS[:, a:b], qs[:, a:b])
                pos += npair
                if i == 1:
                    nc.sync.dma_start(GR[:, :], gr[:, :])
            nc.vector.memset(ZT[:, :], 0.0)
            # guard columns: E_{-1}[j] = [j == 0], O_{-1}[j] = 0
            nc.vector.memset(ES[:, 0:(L + 1) * PW:PW], 0.0)
            nc.vector.memset(OS[:, 0:L * PW:PW], 0.0)
            nc.vector.memset(ES[:, 0:1], 1.0)
            nc.vector.memset(EX[:, 129:132], 0.0)

            for j in range(L + 1):
                ob = (j - 1) * PW
                osh = ZT[:, 0:HT] if j == 0 else OS[:, ob:ob + HT]
                tilted = (j > 0) and (j % TILT_EVERY == 0)
                if tilted:
                    g = GT[(j // TILT_EVERY) % 2]
                    nc.vector.tensor_scalar_mul(g[:, :], osh, GR[:, 0:1])
                    d0e = g[:, 0:HT]
                else:
                    d0e = osh
                eb = j * PW
                nc.vector.tensor_tensor_scan(
                    ES[:, eb + 1:eb + 1 + HT], d0e, QS[:, 0:HT],
                    1.0 if j == 0 else 0.0, ADD, MULT)
                if j == L:
                    break
                b = BT[j % 2]
                nc.vector.scalar_tensor_tensor(
                    b[:, :], osh, QS[:, HT + j:HT + j + 1], ES[:, eb:eb + HT],
                    MULT, ADD)
                nc.vector.tensor_tensor_scan(
                    OS[:, ob + PW + 1:ob + PW + 1 + HT], b[:, 0:HT],
                    QS[:, HDR + j * HT:HDR + (j + 1) * HT], 0.0, ADD, MULT)
                if j == 47:
                    # early export of finished pairs; hides copy + DMA
                    # latency under the remaining chain
                    nc.vector.tensor_copy(EX[:, 0:48], ES[:, HT:48 * PW:PW])
                    nc.vector.tensor_copy(EX[:, L + 1:L + 48],
                                          OS[:, HT:47 * PW:PW])
                    nc.sync.dma_start(afin[:, 0:112], EX[:, 0:112])

            nc.vector.tensor_copy(EX[:, 48:L + 1], ES[:, 48 * PW + HT::PW])
            nc.vector.tensor_copy(EX[:, L + 48:S], OS[:, 47 * PW + HT::PW])
            nc.sync.dma_start(afin[:, 48:132], EX[:, 48:132])
    nc.compile()
    return nc


def _host_prep(y_true, y_pred):
    yt = np.asarray(y_true)
    yp = np.asarray(y_pred, dtype=np.float32)
    pB = yp[:, :, BLANK].astype(np.float64) + EPS            # [B, T]
    pl = (np.take_along_axis(yp, yt[:, None, :].astype(np.int64), axis=2)
          .astype(np.float64) + EPS)                          # [B, T, L]

    # fwd chain (t < HT) and bwd chain (reversed time + labels)
    q_f = pl[:, :HT, :] / pB[:, :HT, None]
    q_b = pl[:, :HT - 1:-1, ::-1] / pB[:, :HT - 1:-1, None]
    K_f = np.zeros((B, L))
    K_f[:, 1:] = (yt[:, 1:] != yt[:, :-1]).astype(np.float64)
    K_b = np.zeros((B, L))
    K_b[:, 1:] = (yt[:, ::-1][:, 1:] != yt[:, ::-1][:, :-1]).astype(np.float64)

    def chain_params(q):
        lnq = np.log(q).mean(axis=(1, 2))
        slope = CF0 * lnq + CF1
        gam = np.exp(-slope)
        r = gam ** (2 * TILT_EVERY)
        # damping from tilted 2-state mean-field surrogate
        e = np.ones((B,)); o = np.zeros((B,))
        g = np.empty((B, HT))
        qb = q.mean(axis=2)
        for t in range(HT):
            e2 = e + gam * o
            o2 = qb[:, t] * (o + gam * e + gam * gam * o)
            z2 = e2 + o2
            g[:, t] = z2 / (e + o)
            e, o = e2 / z2, o2 / z2
        delta = 1.0 / g
        return r, delta

    r_f, d_f = chain_params(q_f)
    r_b, d_b = chain_params(q_b)

    def pack(q, K, r, delta):
        # qs rows: [dl series | K' columns | pair-major q*delta series]
        n = q.shape[0]
        qt = (q * delta[:, :, None]).transpose(0, 2, 1)       # [n, L, HT]
        kc = K.copy()
        for j in range(TILT_EVERY, L, TILT_EVERY):
            kc[:, j] *= r
        qs = np.concatenate(
            [delta, kc, qt.reshape(n, L * HT)], axis=1).astype(bf16)
        return qs, r.astype(np.float32)

    qs_f, gr_f = pack(q_f, K_f, r_f, d_f)
    qs_b, gr_b = pack(q_b, K_b, r_b, d_b)

    in_maps = []
    for ci in range(NCORES):
        ex = slice(ci * BS, (ci + 1) * BS)
        in_maps.append({
            "qs": np.concatenate([qs_f[ex], qs_b[ex]], axis=0),
            "gr": np.concatenate([gr_f[ex], gr_b[ex]], axis=0)[:, None],
        })
    aux = (pB, r_f, r_b, d_f, d_b, yt)
    return in_maps, aux


def _host_combine(afin, aux):
    pB, r_f, r_b, d_f, d_b, yt = aux
    af_s = afin[:, :BS, :].reshape(B, 132).astype(np.float64)
    ab_s = afin[:, BS:, :].reshape(B, 132).astype(np.float64)

    # un-tilt ledger: pair j carries floor(j / TILT_EVERY) factors of r
    nt = np.floor_divide(np.arange(L + 1), TILT_EVERY)
    af = np.zeros((B, S)); ab = np.zeros((B, S))
    af[:, 0::2] = af_s[:, 0:L + 1] * r_f[:, None] ** (-nt[None, :])
    af[:, 1::2] = af_s[:, L + 1:S] * r_f[:, None] ** (-nt[None, :L])
    ab[:, 0::2] = ab_s[:, 0:L + 1] * r_b[:, None] ** (-nt[None, :])
    ab[:, 1::2] = ab_s[:, L + 1:S] * r_b[:, None] ** (-nt[None, :L])

    ext = np.full((B, S), BLANK, np.int64)
    ext[:, 1::2] = yt
    cs = np.zeros((B, S))
    cs[:, 2:] = ((ext[:, 2:] != BLANK)
                 & (ext[:, 2:] != ext[:, :-2])).astype(np.float64)
    zg = np.zeros((B, S + 2))
    zg[:, 2:] = af
    z = zg[:, 2:] + zg[:, 1:-1] + cs * zg[:, 0:-2]
    dot = (z * ab[:, ::-1]).sum(axis=1)

    lnF = np.log(pB).sum(axis=1)
    lnD = np.log(d_f).sum(axis=1) + np.log(d_b).sum(axis=1)
    ll = np.log(np.maximum(dot, 1e-300)) + lnF - lnD
    return (-ll[:, None]).astype(np.float32)


def kernel(y_true, y_pred):
    in_maps, aux = _host_prep(y_true, y_pred)
    if "nc" not in _CACHE:
        _CACHE["nc"] = _build_program()
    nc = _CACHE["nc"]
    res = run_bass_kernel_spmd(nc, in_maps, core_ids=list(range(NCORES)))
    afin = np.stack([np.asarray(res.results[i]["afin"], dtype=np.float32)
                     for i in range(NCORES)])
    return _host_combine(afin, aux)


# revision 12
# speedup vs baseline: 2.2692x; 1.0010x over previous
"""CTC loss (keras ctc_batch_cost semantics) on 8 Trainium2 NeuronCores.

Parity-normalized scan formulation. The CTC extended-state DP alternates
blank (even) and label (odd) states. All even states emit the same blank
probability pB_t, so normalizing the whole state vector by the running
blank product turns every even-state update into a pure shift-add and
every odd-state update into an affine recurrence in the emission RATIO
q_t = pl_t/pB_t. Each state-pair then reduces to three DVE instructions
over the full 256-step time axis:

    scanE:  E_t = delta_t * (E_{t-1} + O[j-1]_{t-1})     (cumsum-scan)
    stt:    b_t = K'_j * O[j-1]_{t-1} + E[j]_{t-1}       (fused mul-add)
    scanO:  O_t = qt[j]_t * (O_{t-1} + b_t)              (affine scan)

where qt = q * delta folds a per-(example,t) damping series delta chosen
from a mean-field surrogate so stored magnitudes stay O(1), and a
per-example tilt r applied every TILT_EVERY pairs (one tensor_scalar plus
host-folded K' = K*r) flattens the exponential state profile so bf16
storage holds the junction products. Forward (t<256) and reverse
(t>=256, states reversed) chains for 64 examples pack the 128 partitions
of each core; the final columns are gathered with two strided copies and
combined on the host in f64 with exact log-corrections for the blank
product, damping, and tilt ledgers.
"""
import numpy as np
import ml_dtypes

import concourse.bass as bass
import concourse.bacc as bacc
import concourse.mybir as mybir
from concourse import tile
from concourse.bass_utils import run_bass_kernel_spmd

B, T, C, L = 512, 512, 128, 64
S = 2 * L + 1
NCORES = 8
BS = B // NCORES        # 64 examples per core
HT = T // 2             # 256 timesteps per chain
BLANK = C - 1
EPS = 1e-7
TILT_EVERY = 8
PW = HT + 1             # per-pair series stride (1 guard col + 256)
QCHUNKS = (1, 3, 8, 8, 8, 8, 8, 8, 8, 4)   # QS DMA chunk sizes in pairs
CF0, CF1 = 1.5689666, 2.17334313   # junction profile slope vs mean ln q

F32 = mybir.dt.float32
BF16 = mybir.dt.bfloat16
ADD = mybir.AluOpType.add
MULT = mybir.AluOpType.mult
bf16 = ml_dtypes.bfloat16

_CACHE = {}


HDR = HT + L            # header cols: dl series + kc columns


def _build_program():
    nc = bacc.Bacc("TRN2", target_bir_lowering=False, debug=False)
    qs = nc.dram_tensor("qs", [128, HDR + L * HT], BF16, kind="ExternalInput")
    gr = nc.dram_tensor("gr", [128, 1], F32, kind="ExternalInput")
    afin = nc.dram_tensor("afin", [128, 132], BF16, kind="ExternalOutput")

    with tile.TileContext(nc) as tc:
        with tc.tile_pool(name="static", bufs=1) as sp:
            ES = sp.tile([128, (L + 1) * PW], BF16)
            OS = sp.tile([128, L * PW], BF16)
            QS = sp.tile([128, HDR + L * HT], BF16)
            ZT = sp.tile([128, HT], BF16)
            GR = sp.tile([128, 1], F32)
            GT = [sp.tile([128, HT], BF16, name=f"gt{i}") for i in range(2)]
            BT = [sp.tile([128, HT], BF16, name=f"bt{i}") for i in range(2)]
            EX = sp.tile([128, 132], BF16)
            # header + first pairs land in chunk 0; gr is not needed until
            # the first tilt hop (pair 8), so its DMA rides later
            pos = 0
            for i, npair in enumerate(QCHUNKS):
                a = 0 if i == 0 else HDR + pos * HT
                b = HDR + (pos + npair) * HT
                nc.sync.dma_start(QS[:, a:b], qs[:, a:b])
                pos += npair
                if i == 1:
                    nc.sync.dma_start(GR[:, :], gr[:, :])
            nc.vector.memset(ZT[:, :], 0.0)
            # guard columns: E_{-1}[j] = [j == 0], O_{-1}[j] = 0
            nc.vector.memset(ES[:, 0:(L + 1) * PW:PW], 0.0)
            nc.vector.memset(OS[:, 0:L * PW:PW], 0.0)
            nc.vector.memset(ES[:, 0:1], 1.0)
            nc.vector.memset(EX[:, 129:132], 0.0)

            for j in range(L + 1):
                ob = (j - 1) * PW
                osh = ZT[:, 0:HT] if j == 0 else OS[:, ob:ob + HT]
                tilted = (j > 0) and (j % TILT_EVERY == 0)
                if tilted:
                    g = GT[(j // TILT_EVERY) % 2]
                    nc.vector.tensor_scalar_mul(g[:, :], osh, GR[:, 0:1])
                    d0e = g[:, 0:HT]
                else:
                    d0e = osh
                eb = j * PW
                nc.vector.tensor_tensor_scan(
                    ES[:, eb + 1:eb + 1 + HT], d0e, QS[:, 0:HT],
                    1.0 if j == 0 else 0.0, ADD, MULT)
                if j == L:
                    break
                b = BT[j % 2]
                nc.vector.scalar_tensor_tensor(
                    b[:, :], osh, QS[:, HT + j:HT + j + 1], ES[:, eb:eb + HT],
                    MULT, ADD)
                nc.vector.tensor_tensor_scan(
                    OS[:, ob + PW + 1:ob + PW + 1 + HT], b[:, 0:HT],
                    QS[:, HDR + j * HT:HDR + (j + 1) * HT], 0.0, ADD, MULT)
            nc.vector.tensor_copy(EX[:, 0:L + 1], ES[:, HT::PW])
            nc.vector.tensor_copy(EX[:, L + 1:S], OS[:, HT::PW])
            nc.sync.dma_start(afin[:, :], EX[:, :])
    nc.compile()
    return nc


def _host_prep(y_true, y_pred):
    yt = np.asarray(y_true)
    yp = np.asarray(y_pred, dtype=np.float32)
    pB = yp[:, :, BLANK].astype(np.float64) + EPS            # [B, T]
    pl = (np.take_along_axis(yp, yt[:, None, :].astype(np.int64), axis=2)
          .astype(np.float64) + EPS)                          # [B, T, L]

    # fwd chain (t < HT) and bwd chain (reversed time + labels)
    q_f = pl[:, :HT, :] / pB[:, :HT, None]
    q_b = pl[:, :HT - 1:-1, ::-1] / pB[:, :HT - 1:-1, None]
    K_f = np.zeros((B, L))
    K_f[:, 1:] = (yt[:, 1:] != yt[:, :-1]).astype(np.float64)
    K_b = np.zeros((B, L))
    K_b[:, 1:] = (yt[:, ::-1][:, 1:] != yt[:, ::-1][:, :-1]).astype(np.float64)

    def chain_params(q):
        lnq = np.log(q).mean(axis=(1, 2))
        slope = CF0 * lnq + CF1
        gam = np.exp(-slope)
        r = gam ** (2 * TILT_EVERY)
        # damping from tilted 2-state mean-field surrogate
        e = np.ones((B,)); o = np.zeros((B,))
        g = np.empty((B, HT))
        qb = q.mean(axis=2)
        for t in range(HT):
            e2 = e + gam * o
            o2 = qb[:, t] * (o + gam * e + gam * gam * o)
            z2 = e2 + o2
            g[:, t] = z2 / (e + o)
            e, o = e2 / z2, o2 / z2
        delta = 1.0 / g
        return r, delta

    r_f, d_f = chain_params(q_f)
    r_b, d_b = chain_params(q_b)

    def pack(q, K, r, delta):
        # qs rows: [dl series | K' columns | pair-major q*delta series]
        n = q.shape[0]
        qt = (q * delta[:, :, None]).transpose(0, 2, 1)       # [n, L, HT]
        kc = K.copy()
        for j in range(TILT_EVERY, L, TILT_EVERY):
            kc[:, j] *= r
        qs = np.concatenate(
            [delta, kc, qt.reshape(n, L * HT)], axis=1).astype(bf16)
        return qs, r.astype(np.float32)

    qs_f, gr_f = pack(q_f, K_f, r_f, d_f)
    qs_b, gr_b = pack(q_b, K_b, r_b, d_b)

    in_maps = []
    for ci in range(NCORES):
        ex = slice(ci * BS, (ci + 1) * BS)
        in_maps.append({
            "qs": np.concatenate([qs_f[ex], qs_b[ex]], axis=0),
            "gr": np.concatenate([gr_f[ex], gr_b[ex]], axis=0)[:, None],
        })
    aux = (pB, r_f, r_b, d_f, d_b, yt)
    return in_maps, aux


def _host_combine(afin, aux):
    pB, r_f, r_b, d_f, d_b, yt = aux
    af_s = afin[:, :BS, :].reshape(B, 132).astype(np.float64)
    ab_s = afin[:, BS:, :].reshape(B, 132).astype(np.float64)

    # un-tilt ledger: pair j carries floor(j / TILT_EVERY) factors of r
    nt = np.floor_divide(np.arange(L + 1), TILT_EVERY)
    af = np.zeros((B, S)); ab = np.zeros((B, S))
    af[:, 0::2] = af_s[:, 0:L + 1] * r_f[:, None] ** (-nt[None, :])
    af[:, 1::2] = af_s[:, L + 1:S] * r_f[:, None] ** (-nt[None, :L])
    ab[:, 0::2] = ab_s[:, 0:L + 1] * r_b[:, None] ** (-nt[None, :])
    ab[:, 1::2] = ab_s[:, L + 1:S] * r_b[:, None] ** (-nt[None, :L])

    ext = np.full((B, S), BLANK, np.int64)
    ext[:, 1::2] = yt
    cs = np.zeros((B, S))
    cs[:, 2:] = ((ext[:, 2:] != BLANK)
                 & (ext[:, 2:] != ext[:, :-2])).astype(np.float64)
    zg = np.zeros((B, S + 2))
    zg[:, 2:] = af
    z = zg[:, 2:] + zg[:, 1:-1] + cs * zg[:, 0:-2]
    dot = (z * ab[:, ::-1]).sum(axis=1)

    lnF = np.log(pB).sum(axis=1)
    lnD = np.log(d_f).sum(axis=1) + np.log(d_b).sum(axis=1)
    ll = np.log(np.maximum(dot, 1e-300)) + lnF - lnD
    return (-ll[:, None]).astype(np.float32)


def kernel(y_true, y_pred):
    in_maps, aux = _host_prep(y_true, y_pred)
    if "nc" not in _CACHE:
        _CACHE["nc"] = _build_program()
    nc = _CACHE["nc"]
    res = run_bass_kernel_spmd(nc, in_maps, core_ids=list(range(NCORES)))
    afin = np.stack([np.asarray(res.results[i]["afin"], dtype=np.float32)
                     for i in range(NCORES)])
    return _host_combine(afin, aux)
